# revision 61
# baseline (speedup 1.0000x reference)
"""BitNetAttention Trainium2 kernel — 8-core SPMD, query-sharded.

Per core c: batch b = c//4, query rows 512*(c%4)..+512. Each core
int4-quantizes its rows, computes qT/kT/v projections (exact small-integer
bf16 matmuls), AllGathers scaled kT/v across its 4-core batch group, runs
full 16-head attention for its 512 queries (scoresT layout, exp on ACT to
float32r, PV matmul with a 65th all-ones lhsT column yielding the softmax
denominator Z exactly), then int8-quant + top-50% sparsify (bisection on
integer levels) + o-projection locally. Host reassembles transposed shards.
"""
import sys
import math

sys.path.insert(0, "/opt/trn_rl_repo")

import numpy as np

B, S, H, NH = 2, 2048, 1024, 16
HD = H // NH          # 64
SHARD = 512           # query rows per core
NCORES = 8
SQRT7 = math.sqrt(7.0)
MAGIC = float(np.float32(3 * 2**22))  # 1.5 * 2^23: RNE rounding magic

_cache = {}


def _build():
    import concourse.bass as bass
    import concourse.bacc as bacc
    import concourse.mybir as mybir
    from concourse.tile import TileContext
    from concourse.masks import make_identity

    dt = mybir.dt
    Alu = mybir.AluOpType
    Act = mybir.ActivationFunctionType
    X = mybir.AxisListType.X

    nc = bacc.Bacc("TRN2", target_bir_lowering=False, debug=False,
                   num_devices=NCORES)

    hs_in = nc.dram_tensor("hs", [SHARD, H], dt.float32, kind="ExternalInput")
    wqT_in = nc.dram_tensor("wqT", [H, H], dt.bfloat16, kind="ExternalInput")
    wkT_in = nc.dram_tensor("wkT", [H, H], dt.bfloat16, kind="ExternalInput")
    wvT_in = nc.dram_tensor("wvT", [H, H], dt.bfloat16, kind="ExternalInput")
    woT_in = nc.dram_tensor("woT", [H, H], dt.float32, kind="ExternalInput")
    cst_in = nc.dram_tensor("cst", [8], dt.float32, kind="ExternalInput")
    outT_out = nc.dram_tensor("outT", [H, SHARD], dt.float32, kind="ExternalOutput")

    gk_src = nc.dram_tensor("gk_src", [H, SHARD], dt.float32r)
    gv_src = nc.dram_tensor("gv_src", [SHARD, H], dt.float32r)
    gk_dst = nc.dram_tensor("gk_dst", [4, H, SHARD], dt.float32r)
    gv_dst = nc.dram_tensor("gv_dst", [4, SHARD, H], dt.float32r)
    arow_q = nc.dram_tensor("arow_q", [SHARD], dt.float32)
    arow_k = nc.dram_tensor("arow_k", [SHARD], dt.float32)
    srow = nc.dram_tensor("srow", [SHARD], dt.float32)

    NT = SHARD // 128   # 4 s-tiles
    HT = H // 128       # 8 h/f/c-tiles
    KT = S // 128       # 16 k-tiles

    with TileContext(nc) as tc:
        with tc.tile_pool(name="base", bufs=1) as bp, \
             tc.tile_pool(name="work", bufs=2) as wp, \
             tc.tile_pool(name="mid", bufs=1) as mp, \
             tc.tile_pool(name="mmps", bufs=2, space="PSUM") as pmm:

            ident = bp.tile([128, 128], dt.float32)
            make_identity(nc, ident[:])
            ones_row = bp.tile([1, 128], dt.float32)
            nc.vector.memset(ones_row[:], 1.0)

            cst_sb = bp.tile([1, 8], dt.float32)
            nc.sync.dma_start(out=cst_sb[:], in_=cst_in[None, :])
            ps_c = pmm.tile([128, 512], dt.float32, tag="mm")
            nc.tensor.matmul(ps_c[:, 0:8], ones_row[:], cst_sb[:], start=True, stop=True)
            cst_bc = bp.tile([128, 8], dt.float32)
            nc.vector.tensor_copy(cst_bc[:], ps_c[:, 0:8])
            AQ8 = cst_bc[:, 0:1]
            AK = cst_bc[:, 1:2]
            AV = cst_bc[:, 2:3]
            AO127 = cst_bc[:, 3:4]

            # persistent across attention:
            qT = mp.tile([128, HT, SHARD], dt.float32r)
            ctx_nat = mp.tile([128, NT, H], dt.float32)

            # ================= phase 1-3: quant + transpose + projections ====
            with tc.tile_pool(name="early", bufs=1) as ep:
                xq = ep.tile([128, NT, H], dt.bfloat16)
                av_cols = ep.tile([128, NT], dt.float32)
                for i in range(NT):
                    hst = wp.tile([128, H], dt.float32, tag="hs")
                    nc.sync.dma_start(out=hst[:], in_=hs_in[i * 128:(i + 1) * 128, :])
                    ssum = wp.tile([128, 1], dt.float32)
                    nc.vector.tensor_reduce(ssum[:], hst[:], axis=X, op=Alu.add,
                                            apply_absolute_value=True)
                    beta = wp.tile([128, 1], dt.float32)
                    nc.vector.tensor_scalar(out=beta[:], in0=ssum[:],
                                            scalar1=float(np.float32(1.0 / H)), scalar2=None,
                                            op0=Alu.mult)
                    denom = wp.tile([128, 1], dt.float32)
                    nc.vector.tensor_scalar(out=denom[:], in0=beta[:],
                                            scalar1=float(np.float32(1e-5)), scalar2=None,
                                            op0=Alu.add)
                    r2 = wp.tile([128, 1], dt.float32)
                    nc.vector.reciprocal(r2[:], denom[:])
                    aqc = wp.tile([128, 1], dt.float32)
                    nc.vector.tensor_scalar(out=aqc[:], in0=beta[:], scalar1=AQ8,
                                            scalar2=None, op0=Alu.mult)
                    nc.sync.dma_start(out=arow_q[i * 128:(i + 1) * 128], in_=aqc[:, 0])
                    akc = wp.tile([128, 1], dt.float32)
                    nc.vector.tensor_scalar(out=akc[:], in0=beta[:], scalar1=AK,
                                            scalar2=None, op0=Alu.mult)
                    nc.sync.dma_start(out=arow_k[i * 128:(i + 1) * 128], in_=akc[:, 0])
                    nc.vector.tensor_scalar(out=av_cols[:, i:i + 1], in0=beta[:],
                                            scalar1=AV, scalar2=None, op0=Alu.mult)
                    y = wp.tile([128, H], dt.float32, tag="y")
                    nc.vector.tensor_scalar(out=y[:], in0=hst[:], scalar1=r2[:],
                                            scalar2=float(np.float32(SQRT7)), op0=Alu.mult,
                                            op1=Alu.mult)
                    nc.vector.tensor_scalar(out=y[:], in0=y[:], scalar1=MAGIC,
                                            scalar2=MAGIC, op0=Alu.add, op1=Alu.subtract)
                    nc.vector.tensor_scalar(out=xq[:, i, :], in0=y[:],
                                            scalar1=float(np.float32(-8.0)),
                                            scalar2=float(np.float32(7.0)),
                                            op0=Alu.max, op1=Alu.min)

                xqT = ep.tile([128, HT, SHARD], dt.bfloat16)
                for i in range(NT):
                    for j in range(HT):
                        nc.sync.dma_start(out=xqT[:, j, i * 128:(i + 1) * 128],
                                          in_=xq[:, i, j * 128:(j + 1) * 128],
                                          transpose=True)

                aq_row = ep.tile([1, SHARD], dt.float32)
                nc.sync.dma_start(out=aq_row[:], in_=arow_q[None, :])
                ak_row = ep.tile([1, SHARD], dt.float32)
                nc.sync.dma_start(out=ak_row[:], in_=arow_k[None, :])
                ps_a = pmm.tile([128, 512], dt.float32, tag="mm")
                nc.tensor.matmul(ps_a[:], ones_row[:], aq_row[:], start=True, stop=True)
                aq_bc = ep.tile([128, SHARD], dt.float32)
                nc.vector.tensor_copy(aq_bc[:], ps_a[:])
                ps_a2 = pmm.tile([128, 512], dt.float32, tag="mm")
                nc.tensor.matmul(ps_a2[:], ones_row[:], ak_row[:], start=True, stop=True)
                ak_bc = ep.tile([128, SHARD], dt.float32)
                nc.vector.tensor_copy(ak_bc[:], ps_a2[:])

                wq_sb = ep.tile([128, HT, H], dt.bfloat16)
                wk_sb = ep.tile([128, HT, H], dt.bfloat16)
                wv_sb = ep.tile([128, HT, H], dt.bfloat16)
                nc.sync.dma_start(out=wq_sb[:], in_=wqT_in.rearrange("(a p) f -> p a f", p=128))
                nc.sync.dma_start(out=wk_sb[:], in_=wkT_in.rearrange("(a p) f -> p a f", p=128))
                nc.sync.dma_start(out=wv_sb[:], in_=wvT_in.rearrange("(a p) f -> p a f", p=128))

                for ft in range(HT):
                    ps = pmm.tile([128, 512], dt.float32, tag="mm")
                    for ht in range(HT):
                        nc.tensor.matmul(ps[:], wq_sb[:, ht, ft * 128:(ft + 1) * 128],
                                         xqT[:, ht, :], start=(ht == 0), stop=(ht == HT - 1))
                    nc.vector.tensor_tensor(out=qT[:, ft, :], in0=ps[:], in1=aq_bc[:],
                                            op=Alu.mult)
                for ft in range(HT):
                    ps = pmm.tile([128, 512], dt.float32, tag="mm")
                    for ht in range(HT):
                        nc.tensor.matmul(ps[:], wk_sb[:, ht, ft * 128:(ft + 1) * 128],
                                         xqT[:, ht, :], start=(ht == 0), stop=(ht == HT - 1))
                    kt_sb = wp.tile([128, 512], dt.float32r, tag="ktw")
                    nc.vector.tensor_tensor(out=kt_sb[:], in0=ps[:], in1=ak_bc[:],
                                            op=Alu.mult)
                    nc.sync.dma_start(
                        out=gk_src.rearrange("(a p) s -> p a s", p=128)[:, ft, :],
                        in_=kt_sb[:])
                for st in range(NT):
                    for fc in range(2):
                        ps = pmm.tile([128, 512], dt.float32, tag="mm")
                        for ht in range(HT):
                            nc.tensor.matmul(ps[:], xqT[:, ht, st * 128:(st + 1) * 128],
                                             wv_sb[:, ht, fc * 512:(fc + 1) * 512],
                                             start=(ht == 0), stop=(ht == HT - 1))
                        vsc = wp.tile([128, 512], dt.float32r, tag="vsc")
                        nc.vector.tensor_scalar(out=vsc[:], in0=ps[:],
                                                scalar1=av_cols[:, st:st + 1],
                                                scalar2=None, op0=Alu.mult)
                        nc.sync.dma_start(
                            out=gv_src[st * 128:(st + 1) * 128, fc * 512:(fc + 1) * 512],
                            in_=vsc[:])

            # ================= phase 4: allgather =================
            RG = [[0, 1, 2, 3], [4, 5, 6, 7]]
            nc.gpsimd.collective_compute("AllGather", Alu.bypass, replica_groups=RG,
                                         ins=[gk_src[:]], outs=[gk_dst[:]])
            nc.gpsimd.collective_compute("AllGather", Alu.bypass, replica_groups=RG,
                                         ins=[gv_src[:]], outs=[gv_dst[:]])

            # ================= phase 5-7: attention =================
            with tc.tile_pool(name="attn", bufs=1) as ap, \
                 tc.tile_pool(name="scps", bufs=2, space="PSUM") as psc, \
                 tc.tile_pool(name="ctxps", bufs=2, space="PSUM") as pcx, \
                 tc.tile_pool(name="probs", bufs=3) as prp, \
                 tc.tile_pool(name="kpair", bufs=2) as kpp:
                vres = ap.tile([128, KT, NH, HD + 1], dt.float32r)
                for ch in range(4):
                    for t in range(4):
                        nc.sync.dma_start(
                            out=vres[:, ch * 4 + t, :, 0:HD],
                            in_=gv_dst[ch][t * 128:(t + 1) * 128, :]
                                .rearrange("p (h d) -> p h d", d=HD))
                ones_f = ap.tile([128, NH], dt.float32)
                nc.vector.memset(ones_f[:], 1.0)
                ones_r = ap.tile([128, NH], dt.float32r)
                nc.vector.tensor_copy(ones_r[:], ones_f[:])
                for t in range(KT):
                    nc.vector.tensor_copy(
                        vres[:, t, :, HD:HD + 1],
                        ones_r.rearrange("p (h o) -> p h o", o=1))

                for pr in range(NH // 2):
                    hA, hB = 2 * pr, 2 * pr + 1
                    kTp = kpp.tile([128, 4, SHARD], dt.float32r, tag="ktp")
                    nc.sync.dma_start(
                        out=kTp[:],
                        in_=gk_dst[:, pr * 128:(pr + 1) * 128, :].rearrange("c p s -> p c s"))
                    pcA = pcx.tile([HD + 1, 512], dt.float32, tag="ctx")
                    pcB = pcx.tile([HD + 1, 512], dt.float32, tag="ctx")
                    for g in range(KT // 2):
                        psA = psc.tile([128, 1024], dt.float32, tag="sc")
                        psB = psc.tile([128, 1024], dt.float32, tag="sc")
                        for gi in range(2):
                            t = 2 * g + gi
                            ksl = kTp[:, t // 4, (t % 4) * 128:(t % 4 + 1) * 128]
                            nc.tensor.matmul(psA[:, gi * 512:(gi + 1) * 512],
                                             ksl[0:64, :], qT[0:64, pr, :],
                                             start=True, stop=True, tile_position=(0, 0))
                            nc.tensor.matmul(psB[:, gi * 512:(gi + 1) * 512],
                                             ksl[64:128, :], qT[64:128, pr, :],
                                             start=True, stop=True, tile_position=(64, 0))
                        pbA = prp.tile([128, 1024], dt.float32r, tag="pb")
                        pbB = prp.tile([128, 1024], dt.float32r, tag="pb")
                        nc.scalar.activation(pbA[:], psA[:], Act.Exp)
                        nc.scalar.activation(pbB[:], psB[:], Act.Exp)
                        for gi in range(2):
                            t = 2 * g + gi
                            nc.tensor.matmul(pcA[:], vres[:, t, hA, :],
                                             pbA[:, gi * 512:(gi + 1) * 512],
                                             start=(t == 0), stop=(t == KT - 1))
                            nc.tensor.matmul(pcB[:], vres[:, t, hB, :],
                                             pbB[:, gi * 512:(gi + 1) * 512],
                                             start=(t == 0), stop=(t == KT - 1))
                    for hh, pc in ((hA, pcA), (hB, pcB)):
                        csb = wp.tile([HD + 1, 512], dt.float32, tag="csb")
                        nc.vector.tensor_copy(csb[:], pc[:])
                        for st in range(NT):
                            pt = pmm.tile([128, 512], dt.float32, tag="mm")
                            nc.tensor.transpose(pt[:, 0:HD + 1],
                                                csb[:, st * 128:(st + 1) * 128],
                                                ident[0:HD + 1, 0:HD + 1])
                            rz = wp.tile([128, 1], dt.float32)
                            nc.vector.reciprocal(rz[:], pt[:, HD:HD + 1])
                            nc.vector.tensor_scalar(
                                out=ctx_nat[:, st, hh * HD:(hh + 1) * HD],
                                in0=pt[:, 0:HD], scalar1=rz[:], scalar2=None,
                                op0=Alu.mult)

            # ================= phase 8: int8 quant + topk =================
            with tc.tile_pool(name="tail", bufs=1) as tp:
                nm = tp.tile([128, NT, H], dt.bfloat16)
                for st in range(NT):
                    cx = ctx_nat[:, st, :]
                    gmax = wp.tile([128, 1], dt.float32)
                    nc.vector.tensor_reduce(gmax[:], cx, axis=X, op=Alu.max,
                                            apply_absolute_value=True)
                    gd = wp.tile([128, 1], dt.float32)
                    nc.vector.tensor_scalar(out=gd[:], in0=gmax[:],
                                            scalar1=float(np.float32(1e-5)),
                                            scalar2=None, op0=Alu.add)
                    rg = wp.tile([128, 1], dt.float32)
                    nc.vector.reciprocal(rg[:], gd[:])
                    sc = wp.tile([128, 1], dt.float32)
                    nc.vector.tensor_scalar(out=sc[:], in0=gmax[:], scalar1=AO127,
                                            scalar2=None, op0=Alu.mult)
                    nc.sync.dma_start(out=srow[st * 128:(st + 1) * 128], in_=sc[:, 0])
                    y = wp.tile([128, H], dt.float32, tag="y")
                    nc.vector.tensor_scalar(out=y[:], in0=cx, scalar1=rg[:],
                                            scalar2=float(np.float32(127.0)), op0=Alu.mult,
                                            op1=Alu.mult)
                    nb = tp.tile([128, H], dt.bfloat16, tag=f"nb{st}")
                    nc.vector.tensor_scalar(out=nb[:], in0=y[:], scalar1=MAGIC,
                                            scalar2=MAGIC, op0=Alu.add, op1=Alu.subtract)
                    ab = tp.tile([128, H], dt.bfloat16, tag=f"ab{st}")
                    nc.vector.scalar_tensor_tensor(out=ab[:], in0=nb[:],
                                                   scalar=-1.0, in1=nb[:],
                                                   op0=Alu.mult, op1=Alu.max)
                    lo = wp.tile([128, 1], dt.float32, tag=f"lo{st}")
                    hi = wp.tile([128, 1], dt.float32, tag=f"hi{st}")
                    nc.vector.memset(lo[:], -1.0)
                    nc.vector.memset(hi[:], 128.0)
                    junk = wp.tile([128, H], dt.bfloat16, tag="junk")
                    for it in range(8):
                        mid = wp.tile([128, 1], dt.float32, tag=f"mid{st}")
                        nc.vector.tensor_tensor(out=mid[:], in0=lo[:], in1=hi[:],
                                                op=Alu.add)
                        nc.vector.tensor_scalar(out=mid[:], in0=mid[:],
                                                scalar1=float(np.float32(0.5)),
                                                scalar2=float(np.float32(-0.25)),
                                                op0=Alu.mult, op1=Alu.add)
                        nc.vector.tensor_scalar(out=mid[:], in0=mid[:], scalar1=MAGIC,
                                                scalar2=MAGIC, op0=Alu.add,
                                                op1=Alu.subtract)
                        cnt = wp.tile([128, 1], dt.float32, tag=f"cnt{st}")
                        nc.vector.scalar_tensor_tensor(out=junk[:], in0=ab[:],
                                                       scalar=mid[:], in1=ab[:],
                                                       op0=Alu.is_le, op1=Alu.bypass,
                                                       accum_out=cnt[:])
                        take = wp.tile([128, 1], dt.uint32, tag=f"tk{st}")
                        nc.vector.tensor_scalar(out=take[:], in0=cnt[:],
                                                scalar1=float(np.float32(512.0)),
                                                scalar2=None, op0=Alu.is_ge)
                        nc.vector.copy_predicated(hi[:], take[:], mid[:])
                        nc.vector.tensor_scalar(out=take[:], in0=cnt[:],
                                                scalar1=float(np.float32(512.0)),
                                                scalar2=None, op0=Alu.is_lt)
                        nc.vector.copy_predicated(lo[:], take[:], mid[:])
                    nc.vector.scalar_tensor_tensor(out=nm[:, st, :], in0=ab[:],
                                                   scalar=hi[:], in1=nb[:],
                                                   op0=Alu.is_ge, op1=Alu.mult)

                # ============== phase 9: transpose + scale + o_proj ==========
                nmT = tp.tile([128, HT, SHARD], dt.bfloat16)
                for st in range(NT):
                    for ct in range(HT):
                        nc.sync.dma_start(out=nmT[:, ct, st * 128:(st + 1) * 128],
                                          in_=nm[:, st, ct * 128:(ct + 1) * 128],
                                          transpose=True)
                sc_row = tp.tile([1, SHARD], dt.float32)
                nc.sync.dma_start(out=sc_row[:], in_=srow[None, :])
                ps_s = pmm.tile([128, 512], dt.float32, tag="mm")
                nc.tensor.matmul(ps_s[:], ones_row[:], sc_row[:], start=True, stop=True)
                sc_bc = tp.tile([128, SHARD], dt.float32)
                nc.vector.tensor_copy(sc_bc[:], ps_s[:])

                rhsT = tp.tile([128, HT, SHARD], dt.float32r)
                for ct in range(HT):
                    nc.vector.tensor_tensor(out=rhsT[:, ct, :], in0=nmT[:, ct, :],
                                            in1=sc_bc[:], op=Alu.mult)

                wo_f = tp.tile([128, HT, H], dt.float32)
                nc.sync.dma_start(out=wo_f[:], in_=woT_in.rearrange("(a p) f -> p a f", p=128))
                wo_r = tp.tile([128, HT, H], dt.float32r)
                nc.vector.tensor_copy(wo_r[:], wo_f[:])
                for ft in range(HT):
                    ps = pmm.tile([128, 512], dt.float32, tag="mm")
                    for ct in range(HT):
                        nc.tensor.matmul(ps[:], wo_r[:, ct, ft * 128:(ft + 1) * 128],
                                         rhsT[:, ct, :], start=(ct == 0),
                                         stop=(ct == HT - 1))
                    ot = wp.tile([128, 512], dt.float32, tag="ot")
                    nc.vector.tensor_copy(ot[:], ps[:])
                    nc.sync.dma_start(out=outT_out[ft * 128:(ft + 1) * 128, :], in_=ot[:])

    nc.compile()
    return nc


def kernel(hidden_states, Wq, Wk, Wv, Wo, sq, sk, sv, so):
    import jax
    import jax.numpy as jnp
    from concourse.bass_utils import run_bass_kernel_spmd
    import ml_dtypes

    cpu = jax.devices("cpu")[0]

    def wquant(W, s):
        with jax.default_device(cpu):
            W32 = np.asarray(W, np.float32)
            w_mean = jnp.mean(jnp.abs(jnp.asarray(W32)))
            w_q = jnp.clip(jnp.round(jnp.asarray(W32) / (w_mean + 1e-5)), -1.0, 1.0)
            return np.asarray(w_q, np.float32), np.float32(np.float32(w_mean) * np.float32(s))

    hidden_states = np.ascontiguousarray(np.asarray(hidden_states, np.float32))
    wq_q, aq = wquant(Wq, np.asarray(sq).reshape(-1)[0])
    wk_q, ak = wquant(Wk, np.asarray(sk).reshape(-1)[0])
    wv_q, av = wquant(Wv, np.asarray(sv).reshape(-1)[0])
    wo_q, ao = wquant(Wo, np.asarray(so).reshape(-1)[0])

    wqT = np.ascontiguousarray(wq_q.T).astype(ml_dtypes.bfloat16)
    wkT = np.ascontiguousarray(wk_q.T).astype(ml_dtypes.bfloat16)
    wvT = np.ascontiguousarray(wv_q.T).astype(ml_dtypes.bfloat16)
    woT = np.ascontiguousarray(wo_q.T).astype(np.float32)

    cst = np.zeros(8, np.float32)
    cst[0] = np.float32(aq / np.float32(math.sqrt(HD)))
    cst[1] = ak
    cst[2] = av
    cst[3] = np.float32(ao / np.float32(127.0))

    if "nc" not in _cache:
        _cache["nc"] = _build()
    nc = _cache["nc"]

    in_maps = []
    for c in range(NCORES):
        b, j = c // 4, c % 4
        in_maps.append({
            "hs": np.ascontiguousarray(hidden_states[b, j * SHARD:(j + 1) * SHARD, :]),
            "wqT": wqT, "wkT": wkT, "wvT": wvT, "woT": woT, "cst": cst,
        })

    _cache["last_in_maps"] = in_maps
    res = run_bass_kernel_spmd(nc, in_maps, list(range(NCORES)))
    _cache["last_res"] = res
    out = np.empty((B, S, H), np.float32)
    for c in range(NCORES):
        b, j = c // 4, c % 4
        out[b, j * SHARD:(j + 1) * SHARD, :] = res.results[c]["outT"].T
    return out

